# revision 18
# baseline (speedup 1.0000x reference)
"""nn_Linear8bit on 8 TRN2 NeuronCores — column-parallel (tensor-parallel on out_features).

out[m, n] = sum_k x[m, k] * wq[n, k] * scale[n] + bias[n]
  x: [2, 512, 4096] f32, wq: [16384, 4096] int32 (int8-valued), scale/bias: [16384] f32

Sharding: W/scale/bias row-sharded 2048/core; x replicated (fed k-major). No collectives.

Host prep (pure layout/bit repack, no arithmetic):
  - x -> x.T [K, M] f32 (k-major replica).
  - wq (int8-valued int32) -> int8, transposed+swizzled to [nt*128, kt, n].
  - scale/bias -> [128, 16] (partition-major per n-tile).

Per-core dataflow (all HWDGE, all casts on DVE, no on-chip transposes):
  - Startup phase covers EIGHT n-tiles x token-chunk-0 (8 live PSUM banks,
    k-major).  x is loaded chunk-0-first in 2-k-tile half-token pieces, so the
    PE does twice the matmul work per arrived x byte and the HBM stream can
    never starve it (x feed 1.4us/piece vs 3.5us PE consumption).
  - W for the startup n-tiles is DMAd and cast in three k-slices (kt 0..3 /
    4..11 / 12..31) so the in-order DVE cast queue delivers each stationary
    just ahead of its first matmul; remaining n-tiles use whole-tile loads
    prefetched on the ACT ring FIFO-behind the x stream (no bandwidth steal).
  - 10 dummy warm-up matmuls on a memset tile keep the PE_HAM clock-gate at
    8/8 through the initial DMA dead time.
  - After the startup phase: chunk-1 of the first 8 n-tiles, then both chunks
    of n-tiles 8..15, all k-inner (stationary reuse across chunks).
  - PSUM evicted via one DVE tensor_scalar (x*scale + bias, per-partition
    scalars); outputs stored as out.T f32 on the SP ring.
  - host: concat core outputs along n, transpose to [1024, 16384].
"""

import numpy as np

import concourse.tile as tile
from concourse import bacc, mybir
from concourse.bass_utils import run_bass_kernel_spmd

B, S, K, N = 2, 512, 4096, 16384
M = B * S              # 1024 tokens
NCORES = 8
NSH = N // NCORES      # 2048 out-features per core
P = 128
KT = K // P            # 32 k-tiles
NT = NSH // P          # 16 n-tiles per core
MCW = 512              # moving free dim per matmul (= one PSUM bank of f32)
MCH = M // MCW         # 2 token chunks
NT_A = 8               # n-tiles in the startup phase (x chunk 0 only)
WS1, WS2 = 4, 12       # startup W tiles cast in kt slices [0,WS1), [WS1,WS2), [WS2,KT)
NDUMMY = 10            # warm-up matmuls
XGK = KT // 2          # 16 x pieces of 2 k-tiles per chunk


def build(w_bufs: int = 3, x_bufs: int = 4, psum_bufs: int = 8):
    nc = bacc.Bacc("TRN2", target_bir_lowering=False, debug=False)
    xT_d = nc.dram_tensor("xT", [K, M], mybir.dt.float32, kind="ExternalInput")
    w_d = nc.dram_tensor("wq", [NT * P, KT, P], mybir.dt.int8, kind="ExternalInput")
    s_d = nc.dram_tensor("scale", [P, NT], mybir.dt.float32, kind="ExternalInput")
    b_d = nc.dram_tensor("bias", [P, NT], mybir.dt.float32, kind="ExternalInput")
    o_d = nc.dram_tensor("outT", [NSH, M], mybir.dt.float32, kind="ExternalOutput")

    with tile.TileContext(nc) as tc:
        with (
            tc.tile_pool(name="xT_pool", bufs=1) as xT_pool,
            tc.tile_pool(name="xstage", bufs=x_bufs) as xstage_pool,
            tc.tile_pool(name="w8s", bufs=4) as w8s_pool,
            tc.tile_pool(name="w8", bufs=2) as w8_pool,
            tc.tile_pool(name="wT_pool", bufs=w_bufs) as wT_pool,
            tc.tile_pool(name="wTabc", bufs=1) as wTabc_pool,
            tc.tile_pool(name="small", bufs=2) as small_pool,
            tc.tile_pool(name="osb", bufs=4) as osb_pool,
            tc.tile_pool(name="psum", bufs=psum_bufs, space="PSUM") as psum_pool,
        ):
            # ---- PE warm-up
            dummy = small_pool.tile([P, MCW], mybir.dt.bfloat16, tag="dummy")
            nc.vector.memset(dummy[:], 0.0)
            psA = [
                psum_pool.tile([P, MCW], mybir.dt.float32, name=f"psA{nt}", tag="ps")
                for nt in range(NT_A)
            ]
            for i in range(NDUMMY):
                nc.tensor.matmul(
                    psA[0][:], dummy[:, 0:P], dummy[:], start=True, stop=True
                )

            # ---- SP ring: startup W pieces (kt slices a/b1/b2 per n-tile)
            w8a, w8b1, w8b2 = {}, {}, {}
            for nt in range(NT_A):
                w8a[nt] = w8s_pool.tile(
                    [P, WS1, P], mybir.dt.int8, name=f"w8a{nt}", tag="w8s"
                )
                nc.sync.dma_start(
                    out=w8a[nt][:], in_=w_d.ap()[nt * P:(nt + 1) * P, 0:WS1]
                )
            for nt in range(NT_A):
                w8b1[nt] = w8s_pool.tile(
                    [P, WS2 - WS1, P], mybir.dt.int8, name=f"w8b1{nt}", tag="w8s"
                )
                nc.sync.dma_start(
                    out=w8b1[nt][:], in_=w_d.ap()[nt * P:(nt + 1) * P, WS1:WS2]
                )
            for nt in range(NT_A):
                w8b2[nt] = w8s_pool.tile(
                    [P, KT - WS2, P], mybir.dt.int8, name=f"w8b2{nt}", tag="w8s"
                )
                nc.sync.dma_start(
                    out=w8b2[nt][:], in_=w_d.ap()[nt * P:(nt + 1) * P, WS2:KT]
                )

            # ---- ACT ring: x chunk-0 pieces, then chunk-1, then scale/bias
            # and the phase-B W tiles (FIFO keeps them out of the x window).
            xstg = [[None] * MCH for _ in range(XGK)]
            for c in range(MCH):
                for g in range(XGK):
                    stg = xstage_pool.tile(
                        [P, 2, MCW], mybir.dt.float32, name=f"xs{g}_{c}", tag="xs"
                    )
                    nc.scalar.dma_start(
                        out=stg[:],
                        in_=xT_d.ap()[
                            2 * g * P:2 * (g + 1) * P, c * MCW:(c + 1) * MCW
                        ].rearrange("(kt p) m -> p kt m", p=P),
                    )
                    xstg[g][c] = stg
            s_sb = small_pool.tile([P, NT], mybir.dt.float32, tag="s_sb")
            nc.scalar.dma_start(out=s_sb[:], in_=s_d.ap())
            b_sb = small_pool.tile([P, NT], mybir.dt.float32, tag="b_sb")
            nc.scalar.dma_start(out=b_sb[:], in_=b_d.ap())

            # ---- DVE cast chain (in-order): startup W slices threaded through
            # the x casts so each arrives just before its first matmul.
            wTa, wTb1, wTb2 = {}, {}, {}
            xTs = [[None] * MCH for _ in range(XGK)]

            def cast_x(g, c):
                xt = xT_pool.tile(
                    [P, 2, MCW], mybir.dt.bfloat16, name=f"xT{g}_{c}",
                    tag=f"xT{g}_{c}"
                )
                nc.vector.tensor_copy(out=xt[:], in_=xstg[g][c][:])
                xTs[g][c] = xt

            def cast_w_slice(dst, src, nt, kn, nm):
                dst[nt] = wTabc_pool.tile(
                    [P, kn, P], mybir.dt.bfloat16, name=f"{nm}{nt}",
                    tag=f"{nm}{nt}"
                )
                nc.vector.tensor_copy(out=dst[nt][:], in_=src[nt][:])

            for nt in range(4):
                cast_w_slice(wTa, w8a, nt, WS1, "wTa")
            cast_x(0, 0)
            for nt in range(4, NT_A):
                cast_w_slice(wTa, w8a, nt, WS1, "wTa")
            cast_x(1, 0)
            cast_w_slice(wTb1, w8b1, 0, WS2 - WS1, "wTb1")
            cast_w_slice(wTb1, w8b1, 1, WS2 - WS1, "wTb1")
            cast_x(2, 0)
            cast_w_slice(wTb1, w8b1, 2, WS2 - WS1, "wTb1")
            cast_w_slice(wTb1, w8b1, 3, WS2 - WS1, "wTb1")
            cast_x(3, 0)
            cast_w_slice(wTb1, w8b1, 4, WS2 - WS1, "wTb1")
            cast_w_slice(wTb1, w8b1, 5, WS2 - WS1, "wTb1")
            cast_x(4, 0)
            cast_w_slice(wTb1, w8b1, 6, WS2 - WS1, "wTb1")
            cast_w_slice(wTb1, w8b1, 7, WS2 - WS1, "wTb1")
            cast_x(5, 0)
            cast_w_slice(wTb2, w8b2, 0, KT - WS2, "wTb2")
            cast_x(6, 0)
            cast_w_slice(wTb2, w8b2, 1, KT - WS2, "wTb2")
            cast_w_slice(wTb2, w8b2, 2, KT - WS2, "wTb2")
            cast_x(7, 0)
            cast_w_slice(wTb2, w8b2, 3, KT - WS2, "wTb2")
            cast_w_slice(wTb2, w8b2, 4, KT - WS2, "wTb2")
            cast_x(8, 0)
            cast_w_slice(wTb2, w8b2, 5, KT - WS2, "wTb2")
            cast_w_slice(wTb2, w8b2, 6, KT - WS2, "wTb2")
            cast_x(9, 0)
            cast_w_slice(wTb2, w8b2, 7, KT - WS2, "wTb2")
            for g in range(10, XGK):
                cast_x(g, 0)
            for g in range(XGK):
                cast_x(g, 1)

            # phase-B W tiles (nt 8..15): DMA on ACT ring (behind x), DVE cast.
            wTs = {}

            def load_w_full(nt):
                w8 = w8_pool.tile([P, KT, P], mybir.dt.int8, name=f"w8_{nt}",
                                  tag="w8")
                nc.scalar.dma_start(out=w8[:], in_=w_d.ap()[nt * P:(nt + 1) * P])
                wT = wT_pool.tile(
                    [P, KT, P], mybir.dt.bfloat16, name=f"wT{nt}", tag="wT"
                )
                nc.vector.tensor_copy(out=wT[:], in_=w8[:])
                wTs[nt] = wT

            for nt in range(NT_A, NT_A + 3):
                load_w_full(nt)

            def stationary(nt, kt):
                if nt < NT_A:
                    if kt < WS1:
                        return wTa[nt][:, kt, :]
                    if kt < WS2:
                        return wTb1[nt][:, kt - WS1, :]
                    return wTb2[nt][:, kt - WS2, :]
                return wTs[nt][:, kt, :]

            def moving(kt, c):
                g, kti = divmod(kt, 2)
                return xTs[g][c][:, kti, :]

            # ---- phase A: chunk 0 of n-tiles 0..7, k-major, 8 live banks
            for kt in range(KT):
                for nt in range(NT_A):
                    nc.tensor.matmul(
                        psA[nt][:], stationary(nt, kt), moving(kt, 0),
                        start=(kt == 0), stop=(kt == KT - 1),
                    )

            def evict(nt, c, ps):
                o_sb = osb_pool.tile(
                    [P, MCW], mybir.dt.float32, name=f"osb{nt}_{c}", tag="o_sb"
                )
                nc.vector.tensor_scalar(
                    out=o_sb[:],
                    in0=ps[:],
                    scalar1=s_sb[:, nt:nt + 1],
                    scalar2=b_sb[:, nt:nt + 1],
                    op0=mybir.AluOpType.mult,
                    op1=mybir.AluOpType.add,
                )
                nc.sync.dma_start(
                    out=o_d.ap()[nt * P:(nt + 1) * P, c * MCW:(c + 1) * MCW],
                    in_=o_sb[:],
                )

            for nt in range(NT_A):
                evict(nt, 0, psA[nt])

            # ---- chunk 1 of n-tiles 0..7, k-inner (stationaries resident)
            for nt in range(NT_A):
                ps = psum_pool.tile(
                    [P, MCW], mybir.dt.float32, name=f"psC{nt}", tag="ps"
                )
                for kt in range(KT):
                    nc.tensor.matmul(
                        ps[:], stationary(nt, kt), moving(kt, 1),
                        start=(kt == 0), stop=(kt == KT - 1),
                    )
                evict(nt, 1, ps)

            # ---- n-tiles 8..15, both chunks, k-inner / ch-inner
            for nt in range(NT_A, NT):
                ps = [
                    psum_pool.tile(
                        [P, MCW], mybir.dt.float32, name=f"ps{nt}_{c}", tag="ps"
                    )
                    for c in range(MCH)
                ]
                for kt in range(KT):
                    for c in range(MCH):
                        nc.tensor.matmul(
                            ps[c][:], stationary(nt, kt), moving(kt, c),
                            start=(kt == 0), stop=(kt == KT - 1),
                        )
                if nt + 3 < NT:
                    load_w_full(nt + 3)
                for c in range(MCH):
                    evict(nt, c, ps[c])

    nc.compile()
    return nc


def make_in_maps(x, weight_quant, scale, bias):
    x2T = np.ascontiguousarray(
        np.asarray(x, dtype=np.float32).reshape(M, K).T
    )  # [K, M] k-major replica
    wq = np.asarray(weight_quant, dtype=np.int32).astype(np.int8)  # int8-valued
    scale = np.asarray(scale, dtype=np.float32)
    bias = np.asarray(bias, dtype=np.float32)
    in_maps = []
    for i in range(NCORES):
        sl = slice(i * NSH, (i + 1) * NSH)
        # [nsh, k] -> [nt, n, kt, p] -> [nt, p, kt, n] -> [nt*p, kt, n]
        w_sw = np.ascontiguousarray(
            wq[sl].reshape(NT, P, KT, P).transpose(0, 3, 2, 1)
        ).reshape(NT * P, KT, P)
        in_maps.append({
            "xT": x2T,
            "wq": w_sw,
            "scale": np.ascontiguousarray(scale[sl].reshape(NT, P).T),
            "bias": np.ascontiguousarray(bias[sl].reshape(NT, P).T),
        })
    return in_maps


def gather_output(results):
    outT = np.concatenate([np.asarray(r["outT"]) for r in results], axis=0)  # [N, M]
    return np.ascontiguousarray(outT.T).reshape(B, S, N).astype(np.float32, copy=False)


def kernel(x, weight_quant, scale, bias):
    nc = build()
    in_maps = make_in_maps(x, weight_quant, scale, bias)
    res = run_bass_kernel_spmd(nc, in_maps, core_ids=list(range(NCORES)))
    return gather_output(res.results)


if __name__ == "__main__":
    rng = np.random.default_rng(0)
    x = rng.standard_normal((B, S, K), dtype=np.float32)
    wq = rng.integers(-128, 128, size=(N, K), dtype=np.int64).astype(np.int32)
    scale = rng.uniform(0.001, 0.02, size=(N,)).astype(np.float32)
    bias = rng.standard_normal((N,)).astype(np.float32)
    out = kernel(x=x, weight_quant=wq, scale=scale, bias=bias)
    w = wq.astype(np.float32) * scale[:, None]
    exp = x.reshape(M, K) @ w.T + bias
    err = np.abs(out.reshape(M, N) - exp).max() / np.abs(exp).max()
    print("self-check rel err:", err)


# revision 22
# speedup vs baseline: 1.0506x; 1.0506x over previous
"""nn_Linear8bit on 8 TRN2 NeuronCores — column-parallel (tensor-parallel on out_features).

out[m, n] = sum_k x[m, k] * wq[n, k] * scale[n] + bias[n]
  x: [2, 512, 4096] f32, wq: [16384, 4096] int32 (int8-valued), scale/bias: [16384] f32

Sharding: W/scale/bias row-sharded 2048/core; x replicated (fed k-major). No collectives.

Host prep (pure layout/bit repack, no arithmetic):
  - x -> x.T [K, M] f32 (k-major replica).
  - wq (int8-valued int32) -> int8, transposed+swizzled to [nt*128, kt, n] so each
    n-tile's stationary block DMAs as contiguous 4KB partition lines.
  - scale/bias -> [128, 16] (partition-major per n-tile).

Per-core dataflow (all HWDGE, no SWDGE cast path, no on-chip transposes):
  - x: f32 DMA on the ACT HWDGE ring (its own ring, fine-grained first pieces so
    the first k-tile lands ~10.5us) -> DVE cast f32->bf16 into resident
    xT[kp, kt, m] tiles (contraction on partitions).
  - W: int8 DMA on the SP HWDGE ring per n-tile -> DVE cast int8->bf16 (int8
    values exact in bf16); first 4 tiles cast in two pieces (kt 0..7 / 8..31)
    so the PE's first stationaries are ready early.
  - ~12 dummy warm-up matmuls on a memset tile run during the initial DMA dead
    time so the PE_HAM clock-gate is at 8/8 when real matmuls start.
  - Startup phase: first 4 n-tiles processed k-group-major with 8 live PSUM
    accumulators while x streams in; steady phase: remaining 12 n-tiles k-inner,
    ch-inner (one stationary per (nt,kt) feeds both 512-token chunks).
  - PSUM evicted via one DVE tensor_scalar (x*scale + bias, per-partition
    scalars); outputs stored as out.T f32 on the SP ring.
  - host: concat core outputs along n, transpose to [1024, 16384].
"""

import numpy as np

import concourse.tile as tile
from concourse import bacc, mybir
from concourse.bass_utils import run_bass_kernel_spmd

B, S, K, N = 2, 512, 4096, 16384
M = B * S              # 1024 tokens
NCORES = 8
NSH = N // NCORES      # 2048 out-features per core
P = 128
KT = K // P            # 32 k-tiles
NT = NSH // P          # 16 n-tiles per core
MCW = 512              # moving free dim per matmul (= one PSUM bank of f32)
MCH = M // MCW         # 2 token chunks
NT_A = 4               # n-tiles processed in the k-group-major startup phase
WSPL = 8               # first-phase W tiles cast in (kt<WSPL, kt>=WSPL) pieces
NDUMMY = 16            # warm-up matmuls (cover the DMA/cast dead time)

# x load piece sizes in k-tiles: small first pieces for fast PE start.
KGS = [1] * 6 + [2] * 13
assert sum(KGS) == KT
KG_START = np.cumsum([0] + KGS).tolist()   # group -> first kt
XG = len(KGS)


def _group_of(kt):
    for g in range(XG):
        if KG_START[g] <= kt < KG_START[g + 1]:
            return g, kt - KG_START[g]
    raise AssertionError


def build(w_bufs: int = 4, x_bufs: int = 4, psum_bufs: int = 8):
    nc = bacc.Bacc("TRN2", target_bir_lowering=False, debug=False)
    xT_d = nc.dram_tensor("xT", [K, M], mybir.dt.float32, kind="ExternalInput")
    w_d = nc.dram_tensor("wq", [NT * P, KT, P], mybir.dt.int8, kind="ExternalInput")
    s_d = nc.dram_tensor("scale", [P, NT], mybir.dt.float32, kind="ExternalInput")
    b_d = nc.dram_tensor("bias", [P, NT], mybir.dt.float32, kind="ExternalInput")
    o_d = nc.dram_tensor("outT", [NSH, M], mybir.dt.float32, kind="ExternalOutput")

    with tile.TileContext(nc) as tc:
        with (
            tc.tile_pool(name="xT_pool", bufs=1) as xT_pool,
            tc.tile_pool(name="xstage", bufs=x_bufs) as xstage_pool,
            tc.tile_pool(name="w8", bufs=w_bufs) as w8_pool,
            tc.tile_pool(name="wT_pool", bufs=w_bufs) as wT_pool,
            tc.tile_pool(name="wTa_pool", bufs=1) as wTa_pool,
            tc.tile_pool(name="wTb_pool", bufs=1) as wTb_pool,
            tc.tile_pool(name="small", bufs=2) as small_pool,
            tc.tile_pool(name="osb", bufs=4) as osb_pool,
            tc.tile_pool(name="psum", bufs=psum_bufs, space="PSUM") as psum_pool,
        ):
            # ---- PE warm-up: dummy matmuls on a zeroed tile during DMA dead time
            dummy = small_pool.tile([P, MCW], mybir.dt.bfloat16, tag="dummy")
            nc.vector.memset(dummy[:], 0.0)

            psA = [
                [
                    psum_pool.tile(
                        [P, MCW], mybir.dt.float32, name=f"psA{nt}_{c}", tag="ps"
                    )
                    for c in range(MCH)
                ]
                for nt in range(NT_A)
            ]
            for i in range(NDUMMY):
                nc.tensor.matmul(
                    psA[0][0][:], dummy[:, 0:P], dummy[:], start=True, stop=True
                )

            # ---- startup DMAs.
            # SP ring: W kt0..WSPL pieces for the first NT_A n-tiles, then the
            # remainders.  ACT ring: x, 1 k-tile at a time (kt0 in two halves).
            w8s = {}
            for nt in range(NT_A):
                w8s[nt] = w8_pool.tile(
                    [P, KT, P], mybir.dt.int8, name=f"w8_{nt}", tag="w8"
                )
                nc.sync.dma_start(
                    out=w8s[nt][:], in_=w_d.ap()[nt * P:(nt + 1) * P]
                )
            xstgs = []
            for g in range(XG):
                xstg = xstage_pool.tile(
                    [P, KGS[g], M], mybir.dt.float32, name=f"xstg{g}", tag="xstg"
                )
                nc.scalar.dma_start(
                    out=xstg[:],
                    in_=xT_d.ap()[
                        KG_START[g] * P:KG_START[g + 1] * P, :
                    ].rearrange("(kt p) m -> p kt m", p=P),
                )
                xstgs.append(xstg)
            # scale/bias on the ACT ring: FIFO-ordered behind the x stream so
            # they can't steal HBM bandwidth from phase A's x feed (needed only
            # at the first evict, long after x is resident).
            s_sb = small_pool.tile([P, NT], mybir.dt.float32, tag="s_sb")
            nc.scalar.dma_start(out=s_sb[:], in_=s_d.ap())
            b_sb = small_pool.tile([P, NT], mybir.dt.float32, tag="b_sb")
            nc.scalar.dma_start(out=b_sb[:], in_=b_d.ap())

            # ---- DVE cast order: W a-pieces and first x pieces interleaved so
            # neither blocks the other's earliest consumer.
            wTa = {}
            wTb = {}
            xTs = [None] * XG

            def cast_x(g):
                xt = xT_pool.tile(
                    [P, KGS[g], M], mybir.dt.bfloat16, name=f"xT{g}", tag=f"xT{g}"
                )
                nc.vector.tensor_copy(out=xt[:], in_=xstgs[g][:])
                xTs[g] = xt

            for nt in range(NT_A):
                wTa[nt] = wTa_pool.tile(
                    [P, WSPL, P], mybir.dt.bfloat16, name=f"wTa{nt}", tag=f"wTa{nt}"
                )
                nc.vector.tensor_copy(out=wTa[nt][:], in_=w8s[nt][:, 0:WSPL, :])
                cast_x(nt)
            for nt in range(NT_A):
                wTb[nt] = wTb_pool.tile(
                    [P, KT - WSPL, P], mybir.dt.bfloat16, name=f"wTb{nt}",
                    tag=f"wTb{nt}"
                )
                nc.vector.tensor_copy(out=wTb[nt][:], in_=w8s[nt][:, WSPL:KT, :])
                cast_x(NT_A + nt)
            for g in range(2 * NT_A, XG):
                cast_x(g)

            def stationary(nt, kt):
                if nt < NT_A:
                    if kt < WSPL:
                        return wTa[nt][:, kt, :]
                    return wTb[nt][:, kt - WSPL, :]
                return wTs[nt][:, kt, :]

            # ---- phase B W prefetch (nt NT_A..NT_A+3): DMA now, cast before
            # the phase-A evicts enter the DVE queue (in-order engine).
            wTs = {}

            def load_w_full(nt):
                # ACT ring, FIFO-behind the x stream: these 512KB prefetches
                # execute only after x is fully resident, so phase A's x feed
                # keeps the whole HBM budget (they're needed at ~67us+).
                w8 = w8_pool.tile([P, KT, P], mybir.dt.int8, name=f"w8_{nt}", tag="w8")
                nc.scalar.dma_start(out=w8[:], in_=w_d.ap()[nt * P:(nt + 1) * P])
                wT = wT_pool.tile(
                    [P, KT, P], mybir.dt.bfloat16, name=f"wT{nt}", tag="wT"
                )
                nc.vector.tensor_copy(out=wT[:], in_=w8[:])
                wTs[nt] = wT

            for nt in range(NT_A, min(NT_A + 4, NT)):
                load_w_full(nt)

            # ---- phase A matmuls: k-group-major across NT_A n-tiles
            for g in range(XG):
                for nt in range(NT_A):
                    for kti in range(KGS[g]):
                        kt = KG_START[g] + kti
                        for c in range(MCH):
                            nc.tensor.matmul(
                                psA[nt][c][:],
                                stationary(nt, kt),
                                xTs[g][:, kti, c * MCW:(c + 1) * MCW],
                                start=(kt == 0),
                                stop=(kt == KT - 1),
                            )

            def evict(nt, c, ps):
                o_sb = osb_pool.tile(
                    [P, MCW], mybir.dt.float32, name=f"osb{nt}_{c}", tag="o_sb"
                )
                nc.vector.tensor_scalar(
                    out=o_sb[:],
                    in0=ps[:],
                    scalar1=s_sb[:, nt:nt + 1],
                    scalar2=b_sb[:, nt:nt + 1],
                    op0=mybir.AluOpType.mult,
                    op1=mybir.AluOpType.add,
                )
                nc.scalar.dma_start(
                    out=o_d.ap()[nt * P:(nt + 1) * P, c * MCW:(c + 1) * MCW],
                    in_=o_sb[:],
                )

            for nt in range(NT_A):
                for c in range(MCH):
                    evict(nt, c, psA[nt][c])

            # ---- phase B: remaining n-tiles, k-inner / ch-inner
            for nt in range(NT_A, NT):
                ps = [
                    psum_pool.tile(
                        [P, MCW], mybir.dt.float32, name=f"ps{nt}_{c}", tag="ps"
                    )
                    for c in range(MCH)
                ]
                for kt in range(KT):
                    g, kti = _group_of(kt)
                    for c in range(MCH):
                        nc.tensor.matmul(
                            ps[c][:],
                            wTs[nt][:, kt, :],
                            xTs[g][:, kti, c * MCW:(c + 1) * MCW],
                            start=(kt == 0),
                            stop=(kt == KT - 1),
                        )
                if nt + 4 < NT:
                    load_w_full(nt + 4)
                for c in range(MCH):
                    evict(nt, c, ps[c])

    nc.compile()
    return nc


def make_in_maps(x, weight_quant, scale, bias):
    x2T = np.ascontiguousarray(
        np.asarray(x, dtype=np.float32).reshape(M, K).T
    )  # [K, M] k-major replica
    wq = np.asarray(weight_quant, dtype=np.int32).astype(np.int8)  # int8-valued
    scale = np.asarray(scale, dtype=np.float32)
    bias = np.asarray(bias, dtype=np.float32)
    in_maps = []
    for i in range(NCORES):
        sl = slice(i * NSH, (i + 1) * NSH)
        # [nsh, k] -> [nt, n, kt, p] -> [nt, p, kt, n] -> [nt*p, kt, n]
        w_sw = np.ascontiguousarray(
            wq[sl].reshape(NT, P, KT, P).transpose(0, 3, 2, 1)
        ).reshape(NT * P, KT, P)
        in_maps.append({
            "xT": x2T,
            "wq": w_sw,
            "scale": np.ascontiguousarray(scale[sl].reshape(NT, P).T),
            "bias": np.ascontiguousarray(bias[sl].reshape(NT, P).T),
        })
    return in_maps


def gather_output(results):
    outT = np.concatenate([np.asarray(r["outT"]) for r in results], axis=0)  # [N, M]
    return np.ascontiguousarray(outT.T).reshape(B, S, N).astype(np.float32, copy=False)


def kernel(x, weight_quant, scale, bias):
    nc = build()
    in_maps = make_in_maps(x, weight_quant, scale, bias)
    res = run_bass_kernel_spmd(nc, in_maps, core_ids=list(range(NCORES)))
    return gather_output(res.results)


if __name__ == "__main__":
    rng = np.random.default_rng(0)
    x = rng.standard_normal((B, S, K), dtype=np.float32)
    wq = rng.integers(-128, 128, size=(N, K), dtype=np.int64).astype(np.int32)
    scale = rng.uniform(0.001, 0.02, size=(N,)).astype(np.float32)
    bias = rng.standard_normal((N,), dtype=np.float32)
    out = kernel(x=x, weight_quant=wq, scale=scale, bias=bias)
    w = wq.astype(np.float32) * scale[:, None]
    exp = x.reshape(M, K) @ w.T + bias
    err = np.abs(out.reshape(M, N) - exp).max() / np.abs(exp).max()
    print("self-check rel err:", err)
